# revision 3
# baseline (speedup 1.0000x reference)
"""Trainium2 Bass kernel for nn_LogisticModel.

Computes, elementwise over [B, T] f32 inputs s, x:
    x_prev[:, t] = x[:, t-1]  (0 for t == 0)
    bias  = sigmoid(gain * s)
    resid = x - decay * x_prev - bias
    logp  = -0.5 * (resid / noise)^2 - (log(noise) + 0.5*log(2*pi))

Data-parallel over the batch axis: each of the 8 NeuronCores processes
B/8 = 512 rows. No cross-core communication (rows are independent).

Memory-bound problem; the rel-err gate (2e-2) leaves room for bf16 I/O,
which halves HBM traffic vs f32: 24 MiB/core -> ~70 us at the ~358 GB/s
per-core HBM limit (measured full-input rel err of this pipeline vs the
f32 oracle: 1.1e-2).  Host casts inputs f32->bf16 and the output back.

Per-core schedule, tiles of [128, W] bf16:
  - ACT (scalar): g = sigmoid(gain*s) in place; q = Square(k*resid),
    k = 1/(noise*sqrt(2)), so q = 0.5*(resid/noise)^2.
  - DVE (vector): t = x + (-decay)*x_prev (1x: shifted view is 2B-
    misaligned); resid = t - g (2x_1p, all-bf16 aligned);
    out = -q - log_norm via tensor_scalar (4x_2p).
  - x is loaded as [128, W+1] with one extra leading column so x and
    x_prev come from one DMA; first column tile instead loads aligned
    and handles t=0 (x_prev = 0) with a 1-column copy.
  - Loads on the SP HWDGE ring; stores on the ACT ring so output stores
    don't head-of-line-block upcoming loads.
"""

import os
import sys
from contextlib import ExitStack

import numpy as np

for _p in ("/root/.axon_site", "/root/.axon_site/_ro/trn_rl_repo",
           "/root/.axon_site/_ro/pypackages", "/opt/trn_rl_repo"):
    if os.path.isdir(_p) and _p not in sys.path:
        sys.path.append(_p)

import ml_dtypes

import concourse.bass as bass
import concourse.bacc as bacc
import concourse.mybir as mybir
import concourse.tile as tile

BF16 = mybir.dt.bfloat16
P = 128

N_CORES = 8
B, T = 4096, 8192

LAST_RESULT = None  # test harness introspection; unused by graders


def build_module(rows, cols, gain, decay, noise, W=2048, load_bufs=6,
                 work_bufs=6, out_bufs=6):
    """Build the single-core Bass module for a [rows, cols] bf16 shard."""
    assert rows % P == 0 and cols % W == 0
    nc = bacc.Bacc()
    s_in = nc.declare_dram_parameter("s", [rows, cols], BF16, isOutput=False)
    x_in = nc.declare_dram_parameter("x", [rows, cols], BF16, isOutput=False)
    out = nc.declare_dram_parameter("out", [rows, cols], BF16, isOutput=True)

    log_norm = float(np.log(noise) + 0.5 * np.log(2.0 * np.pi))
    k = float(np.sqrt(0.5) / noise)  # Square(k*r) = 0.5*(r/noise)^2
    AF = mybir.ActivationFunctionType
    OP = mybir.AluOpType

    with tile.TileContext(nc) as tc, ExitStack() as ctx:
        loads = ctx.enter_context(tc.tile_pool(name="loads", bufs=load_bufs))
        work = ctx.enter_context(tc.tile_pool(name="work", bufs=work_bufs))
        outs = ctx.enter_context(tc.tile_pool(name="outs", bufs=out_bufs))
        n_rb = rows // P
        for rb in range(n_rb):
            r0 = rb * P
            for c0 in range(0, cols, W):
                s_t = loads.tile([P, W], BF16, tag="s")
                nc.sync.dma_start(s_t[:], s_in[r0:r0 + P, c0:c0 + W])
                # bias g = sigmoid(gain * s), in place over s
                nc.scalar.activation(s_t[:], s_t[:], AF.Sigmoid,
                                     scale=float(gain))
                t_t = work.tile([P, W], BF16, tag="t")
                # t = x - decay * x_prev.  x tile carries one extra leading
                # column = x_prev source; a tile must have exactly one DMA
                # producer (STT has room for one sync wait), so the first
                # column tile loads aligned and patches t=0 with a 1-col
                # copy (x_prev = 0 there).
                if c0 == 0:
                    x_t = loads.tile([P, W], BF16, tag="x")
                    nc.sync.dma_start(x_t[:], x_in[r0:r0 + P, 0:W])
                    nc.vector.scalar_tensor_tensor(
                        t_t[:, 1:W], x_t[:, 0:W - 1], -float(decay),
                        x_t[:, 1:W], OP.mult, OP.add)
                    nc.vector.tensor_copy(t_t[:, 0:1], x_t[:, 0:1])
                else:
                    x_t = loads.tile([P, W + 1], BF16, tag="x")
                    nc.sync.dma_start(x_t[:],
                                      x_in[r0:r0 + P, c0 - 1:c0 + W])
                    nc.vector.scalar_tensor_tensor(
                        t_t[:], x_t[:, 0:W], -float(decay),
                        x_t[:, 1:W + 1], OP.mult, OP.add)
                # resid = t - g (2x: all-bf16, aligned, packed)
                nc.vector.tensor_tensor(t_t[:], t_t[:], s_t[:], OP.subtract)
                # q = 0.5*(resid/noise)^2, in place
                nc.scalar.activation(t_t[:], t_t[:], AF.Square, scale=k)
                # out = -q - log_norm (tensor_scalar, 4x_2p)
                o_t = outs.tile([P, W], BF16, tag="o")
                nc.vector.tensor_scalar(o_t[:], t_t[:], -1.0, -log_norm,
                                        OP.mult, OP.add)
                nc.scalar.dma_start(out[r0:r0 + P, c0:c0 + W], o_t[:])
    # Bacc.compile() legalizes sync waits (TRN2: max 1 wait per instruction)
    nc.compile()
    return nc


_MODULE_CACHE = {}


def _get_module(key):
    if key not in _MODULE_CACHE:
        _MODULE_CACHE[key] = build_module(*key)
    return _MODULE_CACHE[key]


def kernel(s, x, gain, decay, noise):
    global LAST_RESULT
    from concourse.bass_utils import run_bass_kernel_spmd

    s = np.asarray(s, dtype=np.float32).astype(ml_dtypes.bfloat16)
    x = np.asarray(x, dtype=np.float32).astype(ml_dtypes.bfloat16)
    b, t = s.shape
    assert b % N_CORES == 0
    rows = b // N_CORES

    nc = _get_module((rows, t, float(gain), float(decay), float(noise)))

    in_maps = [
        {"s": s[i * rows:(i + 1) * rows], "x": x[i * rows:(i + 1) * rows]}
        for i in range(N_CORES)
    ]
    res = run_bass_kernel_spmd(nc, in_maps, list(range(N_CORES)))
    LAST_RESULT = res
    return np.concatenate(
        [res.results[i]["out"] for i in range(N_CORES)],
        axis=0).astype(np.float32)


# revision 4
# speedup vs baseline: 1.1587x; 1.1587x over previous
"""Trainium2 Bass kernel for nn_LogisticModel.

Computes, elementwise over [B, T] f32 inputs s, x:
    x_prev[:, t] = x[:, t-1]  (0 for t == 0)
    bias  = sigmoid(gain * s)
    resid = x - decay * x_prev - bias
    logp  = -0.5 * (resid / noise)^2 - (log(noise) + 0.5*log(2*pi))

Data-parallel over the batch axis: each of the 8 NeuronCores processes
B/8 = 512 rows. No cross-core communication (rows are independent).

Memory-bound problem; the rel-err gate (2e-2) leaves room for reduced-
precision I/O: x and out in bf16, s in fp8 e3m4 (s only feeds the
sigmoid, whose error contribution is tiny) -> 20 MiB/core -> ~59 us at
the ~358 GB/s per-core HBM limit.  Measured full-input rel err of this
exact pipeline vs the f32 oracle: 1.0e-2.  Host casts inputs and casts
the bf16 output back to f32.

Per-core schedule, tiles of [128, W]:
  - ACT (scalar): g = sigmoid(gain*s); q = Square(k*resid) with
    k = 1/(noise*sqrt(2)), i.e. q = 0.5*(resid/noise)^2.
  - DVE (vector): t = x + (-decay)*x_prev (1x: the shifted view is
    2B-misaligned); resid = t - g (2x_1p); out = -q - log_norm via
    tensor_scalar (4x_2p).
  - Emission is software-pipelined with a 3-stage skew (A: load+sig+stt,
    B: tt+square, C: ts+store) so each engine's in-order stream holds
    instructions from different tiles and cross-engine sem waits overlap.
  - Loads on the SP HWDGE ring; stores via GPSIMD SWDGE so the store
    trigger cost stays off the ACT critical path and loads are never
    head-of-line blocked.
  - x tiles carry one extra leading column (= x_prev source) except the
    first column tile, which loads aligned and patches t=0 with a 1-col
    copy (x_prev = 0 there).
"""

import os
import sys
from contextlib import ExitStack

import numpy as np

for _p in ("/root/.axon_site", "/root/.axon_site/_ro/trn_rl_repo",
           "/root/.axon_site/_ro/pypackages", "/opt/trn_rl_repo"):
    if os.path.isdir(_p) and _p not in sys.path:
        sys.path.append(_p)

import ml_dtypes

import concourse.bass as bass
import concourse.bacc as bacc
import concourse.mybir as mybir
import concourse.tile as tile

BF16 = mybir.dt.bfloat16
FP8 = mybir.dt.float8e3  # e3m4: max ~15.9, 4 mantissa bits
P = 128

N_CORES = 8
B, T = 4096, 8192

LAST_RESULT = None  # test harness introspection; unused by graders


def build_module(rows, cols, gain, decay, noise, W=4096, gps_cols=0,
                 gps_store=True, s_bufs=4, x_bufs=6, g_bufs=4, t_bufs=4,
                 o_bufs=3):
    """Build the single-core Bass module for a [rows, cols] shard."""
    assert rows % P == 0 and cols % W == 0
    nc = bacc.Bacc()
    s_in = nc.declare_dram_parameter("s", [rows, cols], FP8, isOutput=False)
    x_in = nc.declare_dram_parameter("x", [rows, cols], BF16, isOutput=False)
    out = nc.declare_dram_parameter("out", [rows, cols], BF16, isOutput=True)

    log_norm = float(np.log(noise) + 0.5 * np.log(2.0 * np.pi))
    k = float(np.sqrt(0.5) / noise)  # Square(k*r) = 0.5*(r/noise)^2
    AF = mybir.ActivationFunctionType
    OP = mybir.AluOpType

    tiles = [(rb * P, c0) for rb in range(rows // P)
             for c0 in range(0, cols, W)]
    n = len(tiles)
    st = {}  # in-flight per-tile SBUF state

    with tile.TileContext(nc) as tc, ExitStack() as ctx:
        pool = ctx.enter_context(tc.tile_pool(name="p", bufs=1))
        # per-tag buffer counts via distinct pools
        sp = ctx.enter_context(tc.tile_pool(name="sp", bufs=s_bufs))
        xp = ctx.enter_context(tc.tile_pool(name="xp", bufs=x_bufs))
        gp = ctx.enter_context(tc.tile_pool(name="gp", bufs=g_bufs))
        tp = ctx.enter_context(tc.tile_pool(name="tp", bufs=t_bufs))
        op_ = ctx.enter_context(tc.tile_pool(name="op", bufs=o_bufs))

        def stage_a(i):
            r0, c0 = tiles[i]
            s_t = sp.tile([P, W], FP8, tag="s")
            nc.sync.dma_start(s_t[:], s_in[r0:r0 + P, c0:c0 + W])
            g_t = gp.tile([P, W], BF16, tag="g")
            nc.scalar.activation(g_t[:], s_t[:], AF.Sigmoid,
                                 scale=float(gain))
            t_t = tp.tile([P, W], BF16, tag="t")
            # t = x - decay * x_prev; split columns DVE / GPSIMD
            cs = W - gps_cols
            if c0 == 0:
                x_t = xp.tile([P, W], BF16, tag="x")
                nc.sync.dma_start(x_t[:], x_in[r0:r0 + P, 0:W])
                nc.vector.scalar_tensor_tensor(
                    t_t[:, 1:cs], x_t[:, 0:cs - 1], -float(decay),
                    x_t[:, 1:cs], OP.mult, OP.add)
                nc.vector.tensor_copy(t_t[:, 0:1], x_t[:, 0:1])
                if gps_cols:
                    nc.gpsimd.scalar_tensor_tensor(
                        t_t[:, cs:W], x_t[:, cs - 1:W - 1], -float(decay),
                        x_t[:, cs:W], OP.mult, OP.add)
            else:
                x_t = xp.tile([P, W + 1], BF16, tag="x")
                nc.sync.dma_start(x_t[:], x_in[r0:r0 + P, c0 - 1:c0 + W])
                nc.vector.scalar_tensor_tensor(
                    t_t[:, 0:cs], x_t[:, 0:cs], -float(decay),
                    x_t[:, 1:cs + 1], OP.mult, OP.add)
                if gps_cols:
                    nc.gpsimd.scalar_tensor_tensor(
                        t_t[:, cs:W], x_t[:, cs:W], -float(decay),
                        x_t[:, cs + 1:W + 1], OP.mult, OP.add)
            st[i] = {"g": g_t, "t": t_t}

        def stage_b(i):
            g_t, t_t = st[i]["g"], st[i]["t"]
            # resid = t - g (2x_1p); q = 0.5*(resid/noise)^2 in place
            nc.vector.tensor_tensor(t_t[:], t_t[:], g_t[:], OP.subtract)
            nc.scalar.activation(t_t[:], t_t[:], AF.Square, scale=k)

        def stage_c(i):
            r0, c0 = tiles[i]
            t_t = st.pop(i)["t"]
            o_t = op_.tile([P, W], BF16, tag="o")
            nc.vector.tensor_scalar(o_t[:], t_t[:], -1.0, -log_norm,
                                    OP.mult, OP.add)
            if gps_store:
                nc.gpsimd.dma_start(out[r0:r0 + P, c0:c0 + W], o_t[:])
            else:
                nc.scalar.dma_start(out[r0:r0 + P, c0:c0 + W], o_t[:])

        for step in range(n + 2):
            if step < n:
                stage_a(step)
            if 1 <= step < n + 1:
                stage_b(step - 1)
            if step >= 2:
                stage_c(step - 2)
    # Bacc.compile() legalizes sync waits (TRN2: max 1 wait per instruction)
    nc.compile()
    return nc


_MODULE_CACHE = {}


def _get_module(key):
    if key not in _MODULE_CACHE:
        _MODULE_CACHE[key] = build_module(*key)
    return _MODULE_CACHE[key]


BUILD_KW = {}  # test-harness override for build experiments


def kernel(s, x, gain, decay, noise):
    global LAST_RESULT
    from concourse.bass_utils import run_bass_kernel_spmd

    s = np.asarray(s, dtype=np.float32).astype(ml_dtypes.float8_e3m4)
    x = np.asarray(x, dtype=np.float32).astype(ml_dtypes.bfloat16)
    b, t = s.shape
    assert b % N_CORES == 0
    rows = b // N_CORES

    key = (rows, t, float(gain), float(decay), float(noise)) + tuple(
        sorted(BUILD_KW.items()))
    if key not in _MODULE_CACHE:
        _MODULE_CACHE[key] = build_module(
            rows, t, float(gain), float(decay), float(noise), **BUILD_KW)
    nc = _MODULE_CACHE[key]

    in_maps = [
        {"s": s[i * rows:(i + 1) * rows], "x": x[i * rows:(i + 1) * rows]}
        for i in range(N_CORES)
    ]
    res = run_bass_kernel_spmd(nc, in_maps, list(range(N_CORES)))
    LAST_RESULT = res
    return np.concatenate(
        [res.results[i]["out"] for i in range(N_CORES)],
        axis=0).astype(np.float32)
